# revision 5
# baseline (speedup 1.0000x reference)
"""Trainium2 Bass kernel for nn_AttentionE.

Computes, per sample i:
    s_i   = sum(d_i)                       # d: (N, 6)
    z_ic  = W * s_i * e_ic + b_c           # e: (N, 5), W scalar, b: (5,)
    a_ic  = exp(tanh(z_ic))
    out_ic = e_ic * a_ic / sum_c(a_ic)     # (eps=1e-7 in ref; negligible, denom >= 5/e)

Sharding: data-parallel over the sample axis across 8 NeuronCores.
On-chip layout: each SBUF partition holds a contiguous run of M samples
(rows stay interleaved, [p, m, c]), so DRAM<->SBUF DMAs are fully
contiguous per partition.

Engine split per tile:
  DVE    : sum-of-6 reduce, z = (W*s) bcast-mul e, sum-of-5 reduce, reciprocal
  ScalarE: tanh (per-component, folds bias b_c), exp
  GpSimd : w = a*e, out = w * r_bcast
"""

import sys

import numpy as np

_REPO = "/opt/trn_rl_repo"
if _REPO not in sys.path:
    sys.path.insert(0, _REPO)

from contextlib import ExitStack, nullcontext

import concourse.bacc as bacc
import concourse.bass as bass
import concourse.tile as tile
from concourse import mybir

N_CORES = 8
N_FULL = 4194304
P = 128  # SBUF partitions

# Tunables
M = 512  # samples per partition per tile
BUFS = 3

# Engine assignment for the multiply stages: "vector" or "gpsimd"
Z_ENGINE = "vector"
W_ENGINE = "gpsimd"
OUT_ENGINE = "gpsimd"
# Number of out-stage components (0..5) computed on DVE instead of OUT_ENGINE,
# to balance the DVE and GpSimd pipeline stages.
OUT_SPLIT_K = 0
# Split the d/e input DMAs into two halves so compute starts earlier.
DMA_SPLIT = False
# Pairwise-add reductions instead of tensor_reduce (fewer DVE cycles).
PAIRWISE = False
# Input-DMA grouping: one d/e DMA covers DMA_GROUP compute sub-tiles
# (bigger transfers, fewer dispatches; compute still pipelines at m).
DMA_GROUP = 1
IN_BUFS = 2  # bufs for the grouped input pools
# Ramp-up: split the first tile into RAMP sub-tiles of m/RAMP samples so the
# pipeline reaches steady state sooner (shorter first serial chain).
RAMP = 1
# Register bias const tiles inside the TileContext (Tile-tracked deps)
# instead of pre-TC memsets + an extra all-engine barrier.
BIAS_IN_TC = False

# test.py can flip this to get profile/exec-time back
TRACE = False
LAST = {}


def build_bass(W: float, bvals, S: int, m: int = M, bufs: int = BUFS, repeats: int = 1):
    """Build the single-core SPMD program: d[S,6], e[S,5] -> out[S,5].

    repeats>1 wraps the whole tile loop in a hardware For_i so bench_loop.py
    can measure steady-state device time via the wall-clock slope over R.
    """
    assert S % (P * m) == 0, (S, P, m)
    T = S // (P * m)
    f32 = mybir.dt.float32
    mult = mybir.AluOpType.mult
    add = mybir.AluOpType.add
    X = mybir.AxisListType.X
    ACT = mybir.ActivationFunctionType

    nc = bacc.Bacc("TRN2", debug=False, num_devices=N_CORES)

    # Register the bias values as const APs so activation(bias=<float>) works.
    for i, v in enumerate(dict.fromkeys(float(x) for x in bvals)):
        t_c = nc.alloc_sbuf_tensor(f"const-bias-{i}", [P, 1], f32)
        nc.gpsimd.memset(t_c.ap(), v)
        nc.const_aps.aps[(f32, v)] = t_c.ap()
    nc.all_engine_barrier()

    d_ap = nc.dram_tensor("d", [S, 6], f32, kind="ExternalInput").ap()
    e_ap = nc.dram_tensor("e", [S, 5], f32, kind="ExternalInput").ap()
    o_ap = nc.dram_tensor("out", [S, 5], f32, kind="ExternalOutput").ap()

    # [T, P, m*c] views; per partition the data is one contiguous DRAM run.
    d_v = d_ap.rearrange("(t p m) c -> t p (m c)", t=T, p=P, m=m)
    e_v = e_ap.rearrange("(t p m) c -> t p (m c)", t=T, p=P, m=m)
    o_v = o_ap.rearrange("(t p m) c -> t p (m c)", t=T, p=P, m=m)

    z_eng = {"vector": nc.vector, "gpsimd": nc.gpsimd}[Z_ENGINE]
    w_eng = {"vector": nc.vector, "gpsimd": nc.gpsimd}[W_ENGINE]
    out_eng = {"vector": nc.vector, "gpsimd": nc.gpsimd}[OUT_ENGINE]

    g = DMA_GROUP
    assert T % g == 0
    if g > 1:
        # grouped views: sample idx = ((tb*P + p)*g + sub)*m + j
        d_vg = d_ap.rearrange("(tb p n) c -> tb p (n c)", tb=T // g, p=P, n=m * g)
        e_vg = e_ap.rearrange("(tb p n) c -> tb p (n c)", tb=T // g, p=P, n=m * g)
        o_vg = o_ap.rearrange(
            "(tb p g m) c -> tb p g (m c)", tb=T // g, p=P, g=g, m=m
        )

    with tile.TileContext(nc) as tc, ExitStack() as ctx:
        if BIAS_IN_TC:
            cpool = ctx.enter_context(tc.tile_pool(name="cpool", bufs=1))
            for i, v in enumerate(dict.fromkeys(float(x) for x in bvals)):
                ct = cpool.tile([P, 1], f32, tag=f"bias{i}")
                nc.gpsimd.memset(ct[:], v)
                nc.const_aps.aps[(f32, v)] = ct[:]

        dpool = ctx.enter_context(
            tc.tile_pool(name="dpool", bufs=IN_BUFS if g > 1 else bufs)
        )
        epool = ctx.enter_context(
            tc.tile_pool(name="epool", bufs=IN_BUFS if g > 1 else bufs)
        )
        zpool = ctx.enter_context(tc.tile_pool(name="zpool", bufs=bufs))
        small = ctx.enter_context(tc.tile_pool(name="small", bufs=bufs))

        def emit(dt_, et, o_dst, mm):
            """Compute + store one sub-tile of mm samples/partition."""
            ev = et.rearrange("p (m c) -> p m c", c=5)
            s_t = small.tile([P, mm], f32, tag="s")
            dv3 = dt_.rearrange("p (m c) -> p m c", c=6)
            nc.vector.tensor_reduce(out=s_t[:], in_=dv3, axis=X, op=add)

            z = zpool.tile([P, 5 * mm], f32, tag="z")
            zv = z[:].rearrange("p (m c) -> p m c", c=5)
            s_b = s_t[:].unsqueeze(-1).broadcast_to([P, mm, 5])
            z_eng.tensor_tensor(out=zv, in0=s_b, in1=ev, op=mult)

            for c in range(5):
                nc.scalar.activation(
                    out=zv[:, :, c],
                    in_=zv[:, :, c],
                    func=ACT.Tanh,
                    bias=float(bvals[c]),
                    scale=float(W),
                )
            nc.scalar.activation(out=z[:], in_=z[:], func=ACT.Exp)

            dnm = small.tile([P, mm], f32, tag="dnm")
            nc.vector.tensor_reduce(out=dnm[:], in_=zv, axis=X, op=add)
            r = small.tile([P, mm], f32, tag="r")
            scr = small.tile([P, mm], f32, tag="scr")
            nc.vector.reciprocal_approx_accurate(out=r[:], in_=dnm[:], scratch=scr[:])

            w_eng.tensor_tensor(out=et, in0=z[:], in1=et, op=mult)
            r_b = r[:].unsqueeze(-1).broadcast_to([P, mm, 5])
            out_eng.tensor_tensor(out=zv, in0=ev, in1=r_b, op=mult)
            nc.sync.dma_start(out=o_dst, in_=z[:])

        rep_ctx = tc.For_i(0, repeats) if repeats > 1 else nullcontext()
        with rep_ctx:
          for t in range(T):
            bt, sub = divmod(t, g)
            if g > 1:
                if sub == 0:
                    dbig = dpool.tile([P, 6 * m * g], f32)
                    nc.sync.dma_start(out=dbig[:], in_=d_vg[bt])
                    ebig = epool.tile([P, 5 * m * g], f32)
                    nc.sync.dma_start(out=ebig[:], in_=e_vg[bt])
                emit(
                    dbig[:, sub * 6 * m : (sub + 1) * 6 * m],
                    ebig[:, sub * 5 * m : (sub + 1) * 5 * m],
                    o_vg[bt][:, sub, :],
                    m,
                )
            elif t == 0 and RAMP > 1:
                mr = m // RAMP
                for k in range(RAMP):
                    dk = dpool.tile([P, 6 * mr], f32, tag="dpool")
                    nc.sync.dma_start(
                        out=dk[:], in_=d_v[0][:, k * 6 * mr : (k + 1) * 6 * mr]
                    )
                    ek = epool.tile([P, 5 * mr], f32, tag="epool")
                    nc.sync.dma_start(
                        out=ek[:], in_=e_v[0][:, k * 5 * mr : (k + 1) * 5 * mr]
                    )
                    emit(
                        dk[:], ek[:], o_v[0][:, k * 5 * mr : (k + 1) * 5 * mr], mr
                    )
            else:
                dt_tile = dpool.tile([P, 6 * m], f32, tag="dpool")
                nc.sync.dma_start(out=dt_tile[:], in_=d_v[t])
                et_tile = epool.tile([P, 5 * m], f32, tag="epool")
                nc.sync.dma_start(out=et_tile[:], in_=e_v[t])
                emit(dt_tile[:], et_tile[:], o_v[t], m)

    # Legalize: split multi-wait instructions (HW allows 1 wait/inst).
    nc.compile()
    return nc


def kernel(d, e, W, b):
    from concourse.bass_utils import run_bass_kernel_spmd

    d = np.ascontiguousarray(d, dtype=np.float32)
    e = np.ascontiguousarray(e, dtype=np.float32)
    n = d.shape[0]
    assert n % N_CORES == 0
    s = n // N_CORES

    nc = build_bass(float(np.asarray(W).reshape(-1)[0]), np.asarray(b).tolist(), s)

    in_maps = [
        {"d": d[i * s : (i + 1) * s], "e": e[i * s : (i + 1) * s]}
        for i in range(N_CORES)
    ]
    res = run_bass_kernel_spmd(nc, in_maps, list(range(N_CORES)), trace=TRACE)
    LAST["results"] = res
    out = np.concatenate([res.results[i]["out"] for i in range(N_CORES)], axis=0)
    return out.astype(np.float32)



# revision 24
# speedup vs baseline: 20.8086x; 20.8086x over previous
"""Trainium2 Bass kernel for nn_AttentionE.

Computes, per sample i:
    s_i   = sum(d_i)                       # d: (N, 6)
    z_ic  = W * s_i * e_ic + b_c           # e: (N, 5), W scalar, b: (5,)
    a_ic  = exp(tanh(z_ic))
    out_ic = e_ic * a_ic / sum_c(a_ic)     # (eps=1e-7 in ref; negligible, denom >= 5/e)

Sharding: data-parallel over the sample axis across 8 NeuronCores.
On-chip layout: each SBUF partition holds a contiguous run of M samples
(rows stay interleaved, [p, m, c]), so DRAM<->SBUF DMAs are fully
contiguous per partition.

Engine split per tile (HW-tuned 2026-08-07; For_i-slope device time
150.7us -> 137.1us per full 8-core pass; DMA-only floor measured
108.6us, so the kernel runs at ~79% of its achievable memory roofline):
  DVE    : sum-of-6 reduce, z = s bcast-mul e, sum-of-5 reduce,
           reciprocal (single-op ~18-bit approx), w = a*e
  ScalarE: tanh (folds scale W + per-component bias b_c), exp;
           also issues the output DMA (separate HWDGE ring from inputs)
  GpSimd : out = w * r_bcast only — Q7 two-input multiplies measured
           ~2x slower than DVE on HW, so Pool keeps a single stage
Pipeline: 4-deep tile pools, first tile split in two (RAMP=2) to fill
the pipeline faster.
"""

import sys

import numpy as np

_REPO = "/opt/trn_rl_repo"
if _REPO not in sys.path:
    sys.path.insert(0, _REPO)

from contextlib import ExitStack, nullcontext

import concourse.bacc as bacc
import concourse.bass as bass
import concourse.tile as tile
from concourse import mybir

N_CORES = 8
N_FULL = 4194304
P = 128  # SBUF partitions

import os as _os

# Tunables (env-overridable for bench sweeps; defaults are the shipped config)
M = int(_os.environ.get("K_M", "512"))  # samples per partition per tile
BUFS = int(_os.environ.get("K_BUFS", "4"))

# Engine assignment for the multiply stages: "vector" or "gpsimd"
Z_ENGINE = _os.environ.get("K_Z_ENGINE", "vector")
W_ENGINE = _os.environ.get("K_W_ENGINE", "vector")
OUT_ENGINE = _os.environ.get("K_OUT_ENGINE", "gpsimd")
# Number of out-stage components (0..5) computed on DVE instead of OUT_ENGINE,
# to balance the DVE and GpSimd pipeline stages.
OUT_SPLIT_K = int(_os.environ.get("K_OUT_SPLIT_K", "0"))
# Split the d/e input DMAs into two halves so compute starts earlier.
DMA_SPLIT = False
# Pairwise-add reductions instead of tensor_reduce (fewer DVE cycles).
PAIRWISE = False
# Input-DMA grouping: one d/e DMA covers DMA_GROUP compute sub-tiles
# (bigger transfers, fewer dispatches; compute still pipelines at m).
DMA_GROUP = int(_os.environ.get("K_DMA_GROUP", "1"))
IN_BUFS = int(_os.environ.get("K_IN_BUFS", "2"))  # bufs for the grouped input pools
# Ramp-up: split the first tile into RAMP sub-tiles of m/RAMP samples so the
# pipeline reaches steady state sooner (shorter first serial chain).
RAMP = int(_os.environ.get("K_RAMP", "2"))
# Register bias const tiles inside the TileContext (Tile-tracked deps)
# instead of pre-TC memsets + an extra all-engine barrier.
BIAS_IN_TC = False
# Engine that issues the out DMA ("sync" = SP ring, "scalar" = ACT ring).
OUT_DMA = _os.environ.get("K_OUT_DMA", "scalar")
# Use the single-op ~18-bit reciprocal instead of the 2-op ~22-bit one.
RECIP_FAST = bool(int(_os.environ.get("K_RECIP_FAST", "1")))
# Split the 6-way d reduction: Pool adds the two halves (3m elems), DVE
# reduces the remaining [m,3] — balances DVE (pipeline pole) against Pool.
POOL_REDUCE6 = bool(int(_os.environ.get("K_POOL_REDUCE6", "0")))

# test.py can flip this to get profile/exec-time back
TRACE = False
LAST = {}

# Diagnostic modes for bench_loop decomposition (never used by kernel()):
# SKIP_COMPUTE: issue only the DMAs (out DMA sends stale z tiles).
# SKIP_DMA: issue only compute (tiles hold stale SBUF data).
import os as _os

SKIP_COMPUTE = bool(int(_os.environ.get("K_SKIP_COMPUTE", "0")))
SKIP_DMA = bool(int(_os.environ.get("K_SKIP_DMA", "0")))


def build_bass(W: float, bvals, S: int, m: int = M, bufs: int = BUFS, repeats: int = 1):
    """Build the single-core SPMD program: d[S,6], e[S,5] -> out[S,5].

    repeats>1 wraps the whole tile loop in a hardware For_i so bench_loop.py
    can measure steady-state device time via the wall-clock slope over R.
    """
    assert S % (P * m) == 0, (S, P, m)
    T = S // (P * m)
    f32 = mybir.dt.float32
    mult = mybir.AluOpType.mult
    add = mybir.AluOpType.add
    X = mybir.AxisListType.X
    ACT = mybir.ActivationFunctionType

    nc = bacc.Bacc("TRN2", debug=False, num_devices=N_CORES)

    # Register the bias values as const APs so activation(bias=<float>) works.
    for i, v in enumerate(dict.fromkeys(float(x) for x in bvals)):
        t_c = nc.alloc_sbuf_tensor(f"const-bias-{i}", [P, 1], f32)
        nc.gpsimd.memset(t_c.ap(), v)
        nc.const_aps.aps[(f32, v)] = t_c.ap()
    nc.all_engine_barrier()

    d_ap = nc.dram_tensor("d", [S, 6], f32, kind="ExternalInput").ap()
    e_ap = nc.dram_tensor("e", [S, 5], f32, kind="ExternalInput").ap()
    o_ap = nc.dram_tensor("out", [S, 5], f32, kind="ExternalOutput").ap()

    # [T, P, m*c] views; per partition the data is one contiguous DRAM run.
    d_v = d_ap.rearrange("(t p m) c -> t p (m c)", t=T, p=P, m=m)
    e_v = e_ap.rearrange("(t p m) c -> t p (m c)", t=T, p=P, m=m)
    o_v = o_ap.rearrange("(t p m) c -> t p (m c)", t=T, p=P, m=m)

    z_eng = {"vector": nc.vector, "gpsimd": nc.gpsimd}[Z_ENGINE]
    w_eng = {"vector": nc.vector, "gpsimd": nc.gpsimd}[W_ENGINE]
    out_eng = {"vector": nc.vector, "gpsimd": nc.gpsimd}[OUT_ENGINE]
    out_dma_eng = {"sync": nc.sync, "scalar": nc.scalar}[OUT_DMA]

    g = DMA_GROUP
    assert T % g == 0
    if g > 1:
        # grouped views: sample idx = ((tb*P + p)*g + sub)*m + j
        d_vg = d_ap.rearrange("(tb p n) c -> tb p (n c)", tb=T // g, p=P, n=m * g)
        e_vg = e_ap.rearrange("(tb p n) c -> tb p (n c)", tb=T // g, p=P, n=m * g)
        o_vg = o_ap.rearrange(
            "(tb p g m) c -> tb p g (m c)", tb=T // g, p=P, g=g, m=m
        )

    with tile.TileContext(nc) as tc, ExitStack() as ctx:
        if BIAS_IN_TC:
            cpool = ctx.enter_context(tc.tile_pool(name="cpool", bufs=1))
            for i, v in enumerate(dict.fromkeys(float(x) for x in bvals)):
                ct = cpool.tile([P, 1], f32, tag=f"bias{i}")
                nc.gpsimd.memset(ct[:], v)
                nc.const_aps.aps[(f32, v)] = ct[:]

        dpool = ctx.enter_context(
            tc.tile_pool(name="dpool", bufs=IN_BUFS if g > 1 else bufs)
        )
        epool = ctx.enter_context(
            tc.tile_pool(name="epool", bufs=IN_BUFS if g > 1 else bufs)
        )
        zpool = ctx.enter_context(tc.tile_pool(name="zpool", bufs=bufs))
        small = ctx.enter_context(tc.tile_pool(name="small", bufs=bufs))

        def emit(dt_, et, o_dst, mm):
            """Compute + store one sub-tile of mm samples/partition."""
            if SKIP_COMPUTE:
                nc.sync.dma_start(out=o_dst, in_=et)
                return
            ev = et.rearrange("p (m c) -> p m c", c=5)
            s_t = small.tile([P, mm], f32, tag="s")
            dv3 = dt_.rearrange("p (m c) -> p m c", c=6)
            if POOL_REDUCE6:
                h3 = small.tile([P, 3 * mm], f32, tag="h3")
                h3v = h3[:].rearrange("p (m c) -> p m c", c=3)
                nc.gpsimd.tensor_tensor(
                    out=h3v, in0=dv3[:, :, 0:3], in1=dv3[:, :, 3:6], op=add
                )
                nc.vector.tensor_reduce(out=s_t[:], in_=h3v, axis=X, op=add)
            else:
                nc.vector.tensor_reduce(out=s_t[:], in_=dv3, axis=X, op=add)

            z = zpool.tile([P, 5 * mm], f32, tag="z")
            zv = z[:].rearrange("p (m c) -> p m c", c=5)
            s_b = s_t[:].unsqueeze(-1).broadcast_to([P, mm, 5])
            z_eng.tensor_tensor(out=zv, in0=s_b, in1=ev, op=mult)

            for c in range(5):
                nc.scalar.activation(
                    out=zv[:, :, c],
                    in_=zv[:, :, c],
                    func=ACT.Tanh,
                    bias=float(bvals[c]),
                    scale=float(W),
                )
            nc.scalar.activation(out=z[:], in_=z[:], func=ACT.Exp)

            dnm = small.tile([P, mm], f32, tag="dnm")
            nc.vector.tensor_reduce(out=dnm[:], in_=zv, axis=X, op=add)
            r = small.tile([P, mm], f32, tag="r")
            if RECIP_FAST:
                # ~51 ULP (≈18 bits): orders of magnitude inside the 2e-2 gate.
                nc.vector.reciprocal_approx_fast(out=r[:], in_=dnm[:])
            else:
                scr = small.tile([P, mm], f32, tag="scr")
                nc.vector.reciprocal_approx_accurate(
                    out=r[:], in_=dnm[:], scratch=scr[:]
                )

            w_eng.tensor_tensor(out=et, in0=z[:], in1=et, op=mult)
            r_b = r[:].unsqueeze(-1).broadcast_to([P, mm, 5])
            k = OUT_SPLIT_K
            if k > 0:
                nc.vector.tensor_tensor(
                    out=zv[:, :, :k], in0=ev[:, :, :k], in1=r_b[:, :, :k], op=mult
                )
                out_eng.tensor_tensor(
                    out=zv[:, :, k:], in0=ev[:, :, k:], in1=r_b[:, :, k:], op=mult
                )
            else:
                out_eng.tensor_tensor(out=zv, in0=ev, in1=r_b, op=mult)
            if not SKIP_DMA:
                out_dma_eng.dma_start(out=o_dst, in_=z[:])

        if SKIP_DMA:
            assert g == 1 and RAMP == 1
            for _ in range(bufs):
                dt0 = dpool.tile([P, 6 * m], f32, tag="dpool")
                nc.vector.memset(dt0[:], 0.0)
                et0 = epool.tile([P, 5 * m], f32, tag="epool")
                nc.vector.memset(et0[:], 0.0)

        rep_ctx = tc.For_i(0, repeats) if repeats > 1 else nullcontext()
        with rep_ctx:
          for t in range(T):
            bt, sub = divmod(t, g)
            if g > 1:
                if sub == 0:
                    dbig = dpool.tile([P, 6 * m * g], f32)
                    nc.sync.dma_start(out=dbig[:], in_=d_vg[bt])
                    ebig = epool.tile([P, 5 * m * g], f32)
                    nc.sync.dma_start(out=ebig[:], in_=e_vg[bt])
                emit(
                    dbig[:, sub * 6 * m : (sub + 1) * 6 * m],
                    ebig[:, sub * 5 * m : (sub + 1) * 5 * m],
                    o_vg[bt][:, sub, :],
                    m,
                )
            elif t == 0 and RAMP > 1:
                mr = m // RAMP
                for k in range(RAMP):
                    dk = dpool.tile([P, 6 * mr], f32, tag="dpool")
                    nc.sync.dma_start(
                        out=dk[:], in_=d_v[0][:, k * 6 * mr : (k + 1) * 6 * mr]
                    )
                    ek = epool.tile([P, 5 * mr], f32, tag="epool")
                    nc.sync.dma_start(
                        out=ek[:], in_=e_v[0][:, k * 5 * mr : (k + 1) * 5 * mr]
                    )
                    emit(
                        dk[:], ek[:], o_v[0][:, k * 5 * mr : (k + 1) * 5 * mr], mr
                    )
            else:
                dt_tile = dpool.tile([P, 6 * m], f32, tag="dpool")
                et_tile = epool.tile([P, 5 * m], f32, tag="epool")
                if not SKIP_DMA:
                    nc.sync.dma_start(out=dt_tile[:], in_=d_v[t])
                    nc.sync.dma_start(out=et_tile[:], in_=e_v[t])
                emit(dt_tile[:], et_tile[:], o_v[t], m)

    # Legalize: split multi-wait instructions (HW allows 1 wait/inst).
    nc.compile()
    return nc


def kernel(d, e, W, b):
    from concourse.bass_utils import run_bass_kernel_spmd

    d = np.ascontiguousarray(d, dtype=np.float32)
    e = np.ascontiguousarray(e, dtype=np.float32)
    n = d.shape[0]
    assert n % N_CORES == 0
    s = n // N_CORES

    nc = build_bass(float(np.asarray(W).reshape(-1)[0]), np.asarray(b).tolist(), s)

    in_maps = [
        {"d": d[i * s : (i + 1) * s], "e": e[i * s : (i + 1) * s]}
        for i in range(N_CORES)
    ]
    res = run_bass_kernel_spmd(nc, in_maps, list(range(N_CORES)), trace=TRACE)
    LAST["results"] = res
    out = np.concatenate([res.results[i]["out"] for i in range(N_CORES)], axis=0)
    return out.astype(np.float32)



# revision 25
# speedup vs baseline: 28.7935x; 1.3837x over previous
"""Trainium2 Bass kernel for nn_AttentionE.

Computes, per sample i:
    s_i   = sum(d_i)                       # d: (N, 6)
    z_ic  = W * s_i * e_ic + b_c           # e: (N, 5), W scalar, b: (5,)
    a_ic  = exp(tanh(z_ic))
    out_ic = e_ic * a_ic / sum_c(a_ic)     # (eps=1e-7 in ref; negligible, denom >= 5/e)

Sharding: data-parallel over the sample axis across 8 NeuronCores.
On-chip layout: each SBUF partition holds a contiguous run of M samples
(rows stay interleaved, [p, m, c]), so DRAM<->SBUF DMAs are fully
contiguous per partition.

Engine split per tile (HW-tuned 2026-08-07; For_i-slope device time
150.7us -> 137.1us per full 8-core pass; DMA-only floor measured
108.6us, so the kernel runs at ~79% of its achievable memory roofline):
  DVE    : sum-of-6 reduce, z = s bcast-mul e, sum-of-5 reduce,
           reciprocal (single-op ~18-bit approx), w = a*e
  ScalarE: tanh (folds scale W + per-component bias b_c), exp;
           also issues the output DMA (separate HWDGE ring from inputs)
  GpSimd : out = w * r_bcast only — Q7 two-input multiplies measured
           ~2x slower than DVE on HW, so Pool keeps a single stage
Pipeline: 4-deep tile pools, first tile split in two (RAMP=2) to fill
the pipeline faster.
"""

import sys

import numpy as np

_REPO = "/opt/trn_rl_repo"
if _REPO not in sys.path:
    sys.path.insert(0, _REPO)

from contextlib import ExitStack, nullcontext

import concourse.bacc as bacc
import concourse.bass as bass
import concourse.tile as tile
from concourse import mybir

N_CORES = 8
N_FULL = 4194304
P = 128  # SBUF partitions

import os as _os

# Tunables (env-overridable for bench sweeps; defaults are the shipped config)
M = int(_os.environ.get("K_M", "512"))  # samples per partition per tile
BUFS = int(_os.environ.get("K_BUFS", "4"))

# Engine assignment for the multiply stages: "vector" or "gpsimd"
Z_ENGINE = _os.environ.get("K_Z_ENGINE", "vector")
W_ENGINE = _os.environ.get("K_W_ENGINE", "vector")
OUT_ENGINE = _os.environ.get("K_OUT_ENGINE", "gpsimd")
# Number of out-stage components (0..5) computed on DVE instead of OUT_ENGINE,
# to balance the DVE and GpSimd pipeline stages.
OUT_SPLIT_K = int(_os.environ.get("K_OUT_SPLIT_K", "0"))
# Split the d/e input DMAs into two halves so compute starts earlier.
DMA_SPLIT = False
# Pairwise-add reductions instead of tensor_reduce (fewer DVE cycles).
PAIRWISE = False
# Input-DMA grouping: one d/e DMA covers DMA_GROUP compute sub-tiles
# (bigger transfers, fewer dispatches; compute still pipelines at m).
DMA_GROUP = int(_os.environ.get("K_DMA_GROUP", "1"))
IN_BUFS = int(_os.environ.get("K_IN_BUFS", "2"))  # bufs for the grouped input pools
# Ramp-up: split the first tile into RAMP sub-tiles of m/RAMP samples so the
# pipeline reaches steady state sooner (shorter first serial chain).
RAMP = int(_os.environ.get("K_RAMP", "2"))
# Register bias const tiles inside the TileContext (Tile-tracked deps)
# instead of pre-TC memsets + an extra all-engine barrier.
BIAS_IN_TC = False
# Engine that issues the out DMA ("sync" = SP ring, "scalar" = ACT ring).
OUT_DMA = _os.environ.get("K_OUT_DMA", "scalar")
# Use the single-op ~18-bit reciprocal instead of the 2-op ~22-bit one.
RECIP_FAST = bool(int(_os.environ.get("K_RECIP_FAST", "1")))
# Split the 6-way d reduction: Pool adds the two halves (3m elems), DVE
# reduces the remaining [m,3] — balances DVE (pipeline pole) against Pool.
POOL_REDUCE6 = bool(int(_os.environ.get("K_POOL_REDUCE6", "0")))

# test.py can flip this to get profile/exec-time back
TRACE = False
LAST = {}

# Diagnostic modes for bench_loop decomposition (never used by kernel()):
# SKIP_COMPUTE: issue only the DMAs (out DMA sends stale e tiles).
# SKIP_DMA: issue only compute (tiles hold stale SBUF data).
SKIP_COMPUTE = bool(int(_os.environ.get("K_SKIP_COMPUTE", "0")))
SKIP_DMA = bool(int(_os.environ.get("K_SKIP_DMA", "0")))


def build_bass(W: float, bvals, S: int, m: int = M, bufs: int = BUFS, repeats: int = 1):
    """Build the single-core SPMD program: d[S,6], e[S,5] -> out[S,5].

    repeats>1 wraps the whole tile loop in a hardware For_i so bench_loop.py
    can measure steady-state device time via the wall-clock slope over R.
    """
    assert S % (P * m) == 0, (S, P, m)
    T = S // (P * m)
    f32 = mybir.dt.float32
    mult = mybir.AluOpType.mult
    add = mybir.AluOpType.add
    X = mybir.AxisListType.X
    ACT = mybir.ActivationFunctionType

    nc = bacc.Bacc("TRN2", debug=False, num_devices=N_CORES)

    # Register the bias values as const APs so activation(bias=<float>) works.
    for i, v in enumerate(dict.fromkeys(float(x) for x in bvals)):
        t_c = nc.alloc_sbuf_tensor(f"const-bias-{i}", [P, 1], f32)
        nc.gpsimd.memset(t_c.ap(), v)
        nc.const_aps.aps[(f32, v)] = t_c.ap()
    nc.all_engine_barrier()

    d_ap = nc.dram_tensor("d", [S, 6], f32, kind="ExternalInput").ap()
    e_ap = nc.dram_tensor("e", [S, 5], f32, kind="ExternalInput").ap()
    o_ap = nc.dram_tensor("out", [S, 5], f32, kind="ExternalOutput").ap()

    # [T, P, m*c] views; per partition the data is one contiguous DRAM run.
    d_v = d_ap.rearrange("(t p m) c -> t p (m c)", t=T, p=P, m=m)
    e_v = e_ap.rearrange("(t p m) c -> t p (m c)", t=T, p=P, m=m)
    o_v = o_ap.rearrange("(t p m) c -> t p (m c)", t=T, p=P, m=m)

    z_eng = {"vector": nc.vector, "gpsimd": nc.gpsimd}[Z_ENGINE]
    w_eng = {"vector": nc.vector, "gpsimd": nc.gpsimd}[W_ENGINE]
    out_eng = {"vector": nc.vector, "gpsimd": nc.gpsimd}[OUT_ENGINE]
    out_dma_eng = {"sync": nc.sync, "scalar": nc.scalar}[OUT_DMA]

    g = DMA_GROUP
    assert T % g == 0
    if g > 1:
        # grouped views: sample idx = ((tb*P + p)*g + sub)*m + j
        d_vg = d_ap.rearrange("(tb p n) c -> tb p (n c)", tb=T // g, p=P, n=m * g)
        e_vg = e_ap.rearrange("(tb p n) c -> tb p (n c)", tb=T // g, p=P, n=m * g)
        o_vg = o_ap.rearrange(
            "(tb p g m) c -> tb p g (m c)", tb=T // g, p=P, g=g, m=m
        )

    with tile.TileContext(nc) as tc, ExitStack() as ctx:
        if BIAS_IN_TC:
            cpool = ctx.enter_context(tc.tile_pool(name="cpool", bufs=1))
            for i, v in enumerate(dict.fromkeys(float(x) for x in bvals)):
                ct = cpool.tile([P, 1], f32, tag=f"bias{i}")
                nc.gpsimd.memset(ct[:], v)
                nc.const_aps.aps[(f32, v)] = ct[:]

        dpool = ctx.enter_context(
            tc.tile_pool(name="dpool", bufs=IN_BUFS if g > 1 else bufs)
        )
        epool = ctx.enter_context(
            tc.tile_pool(name="epool", bufs=IN_BUFS if g > 1 else bufs)
        )
        zpool = ctx.enter_context(tc.tile_pool(name="zpool", bufs=bufs))
        small = ctx.enter_context(tc.tile_pool(name="small", bufs=bufs))

        def emit(dt_, et, o_dst, mm):
            """Compute + store one sub-tile of mm samples/partition."""
            if SKIP_COMPUTE:
                nc.sync.dma_start(out=o_dst, in_=et)
                return
            ev = et.rearrange("p (m c) -> p m c", c=5)
            s_t = small.tile([P, mm], f32, tag="s")
            dv3 = dt_.rearrange("p (m c) -> p m c", c=6)
            if POOL_REDUCE6:
                h3 = small.tile([P, 3 * mm], f32, tag="h3")
                h3v = h3[:].rearrange("p (m c) -> p m c", c=3)
                nc.gpsimd.tensor_tensor(
                    out=h3v, in0=dv3[:, :, 0:3], in1=dv3[:, :, 3:6], op=add
                )
                nc.vector.tensor_reduce(out=s_t[:], in_=h3v, axis=X, op=add)
            else:
                nc.vector.tensor_reduce(out=s_t[:], in_=dv3, axis=X, op=add)

            z = zpool.tile([P, 5 * mm], f32, tag="z")
            zv = z[:].rearrange("p (m c) -> p m c", c=5)
            s_b = s_t[:].unsqueeze(-1).broadcast_to([P, mm, 5])
            z_eng.tensor_tensor(out=zv, in0=s_b, in1=ev, op=mult)

            for c in range(5):
                nc.scalar.activation(
                    out=zv[:, :, c],
                    in_=zv[:, :, c],
                    func=ACT.Tanh,
                    bias=float(bvals[c]),
                    scale=float(W),
                )
            nc.scalar.activation(out=z[:], in_=z[:], func=ACT.Exp)

            dnm = small.tile([P, mm], f32, tag="dnm")
            nc.vector.tensor_reduce(out=dnm[:], in_=zv, axis=X, op=add)
            r = small.tile([P, mm], f32, tag="r")
            if RECIP_FAST:
                # ~51 ULP (≈18 bits): orders of magnitude inside the 2e-2 gate.
                nc.vector.reciprocal_approx_fast(out=r[:], in_=dnm[:])
            else:
                scr = small.tile([P, mm], f32, tag="scr")
                nc.vector.reciprocal_approx_accurate(
                    out=r[:], in_=dnm[:], scratch=scr[:]
                )

            w_eng.tensor_tensor(out=et, in0=z[:], in1=et, op=mult)
            r_b = r[:].unsqueeze(-1).broadcast_to([P, mm, 5])
            k = OUT_SPLIT_K
            if k > 0:
                nc.vector.tensor_tensor(
                    out=zv[:, :, :k], in0=ev[:, :, :k], in1=r_b[:, :, :k], op=mult
                )
                out_eng.tensor_tensor(
                    out=zv[:, :, k:], in0=ev[:, :, k:], in1=r_b[:, :, k:], op=mult
                )
            else:
                out_eng.tensor_tensor(out=zv, in0=ev, in1=r_b, op=mult)
            if not SKIP_DMA:
                out_dma_eng.dma_start(out=o_dst, in_=z[:])

        if SKIP_DMA:
            assert g == 1 and RAMP == 1
            for _ in range(bufs):
                dt0 = dpool.tile([P, 6 * m], f32, tag="dpool")
                nc.vector.memset(dt0[:], 0.0)
                et0 = epool.tile([P, 5 * m], f32, tag="epool")
                nc.vector.memset(et0[:], 0.0)

        rep_ctx = tc.For_i(0, repeats) if repeats > 1 else nullcontext()
        with rep_ctx:
          for t in range(T):
            bt, sub = divmod(t, g)
            if g > 1:
                if sub == 0:
                    dbig = dpool.tile([P, 6 * m * g], f32)
                    nc.sync.dma_start(out=dbig[:], in_=d_vg[bt])
                    ebig = epool.tile([P, 5 * m * g], f32)
                    nc.sync.dma_start(out=ebig[:], in_=e_vg[bt])
                emit(
                    dbig[:, sub * 6 * m : (sub + 1) * 6 * m],
                    ebig[:, sub * 5 * m : (sub + 1) * 5 * m],
                    o_vg[bt][:, sub, :],
                    m,
                )
            elif t == 0 and RAMP > 1:
                mr = m // RAMP
                for k in range(RAMP):
                    dk = dpool.tile([P, 6 * mr], f32, tag="dpool")
                    nc.sync.dma_start(
                        out=dk[:], in_=d_v[0][:, k * 6 * mr : (k + 1) * 6 * mr]
                    )
                    ek = epool.tile([P, 5 * mr], f32, tag="epool")
                    nc.sync.dma_start(
                        out=ek[:], in_=e_v[0][:, k * 5 * mr : (k + 1) * 5 * mr]
                    )
                    emit(
                        dk[:], ek[:], o_v[0][:, k * 5 * mr : (k + 1) * 5 * mr], mr
                    )
            else:
                dt_tile = dpool.tile([P, 6 * m], f32, tag="dpool")
                et_tile = epool.tile([P, 5 * m], f32, tag="epool")
                if not SKIP_DMA:
                    nc.sync.dma_start(out=dt_tile[:], in_=d_v[t])
                    nc.sync.dma_start(out=et_tile[:], in_=e_v[t])
                emit(dt_tile[:], et_tile[:], o_v[t], m)

    # Legalize: split multi-wait instructions (HW allows 1 wait/inst).
    nc.compile()
    return nc


def kernel(d, e, W, b):
    from concourse.bass_utils import run_bass_kernel_spmd

    d = np.ascontiguousarray(d, dtype=np.float32)
    e = np.ascontiguousarray(e, dtype=np.float32)
    n = d.shape[0]
    assert n % N_CORES == 0
    s = n // N_CORES

    nc = build_bass(float(np.asarray(W).reshape(-1)[0]), np.asarray(b).tolist(), s)

    in_maps = [
        {"d": d[i * s : (i + 1) * s], "e": e[i * s : (i + 1) * s]}
        for i in range(N_CORES)
    ]
    res = run_bass_kernel_spmd(nc, in_maps, list(range(N_CORES)), trace=TRACE)
    LAST["results"] = res
    out = np.concatenate([res.results[i]["out"] for i in range(N_CORES)], axis=0)
    return out.astype(np.float32)



# revision 30
# speedup vs baseline: 32.3601x; 1.1239x over previous
"""Trainium2 Bass kernel for nn_AttentionE.

Computes, per sample i:
    s_i   = sum(d_i)                       # d: (N, 6)
    z_ic  = W * s_i * e_ic + b_c           # e: (N, 5), W scalar, b: (5,)
    a_ic  = exp(tanh(z_ic))
    out_ic = e_ic * a_ic / sum_c(a_ic)     # (eps=1e-7 in ref; negligible, denom >= 5/e)

Sharding: data-parallel over the sample axis across 8 NeuronCores.
On-chip layout: each SBUF partition holds a contiguous run of M samples
(rows stay interleaved, [p, m, c]), so DRAM<->SBUF DMAs are fully
contiguous per partition.

Engine split per tile (HW-tuned 2026-08-07; For_i-slope device time
150.7us -> 126.1us per full 8-core pass; DMA-only floor measured
108.6us, so the kernel runs at ~86% of its achievable memory roofline):
  DVE    : sum-of-6 reduce, z = s bcast-mul e, sum-of-5 reduce,
           reciprocal (single-op ~18-bit approx), 3/5 of out = w*r_bcast
  ScalarE: tanh (folds scale W + per-component bias b_c), exp;
           also issues the output DMA (separate HWDGE ring from inputs)
  GpSimd : w = a*e and the remaining 2/5 of out = w*r_bcast — Q7
           two-input multiplies run ~2x slower than DVE on HW, so the
           DVE/Pool balance (not exiling Pool) is what the sweep favors
Pipeline: 4-deep tile pools, first tile split in two (RAMP=2) to fill
the pipeline faster.
"""

import sys

import numpy as np

_REPO = "/opt/trn_rl_repo"
if _REPO not in sys.path:
    sys.path.insert(0, _REPO)

from contextlib import ExitStack, nullcontext

import concourse.bacc as bacc
import concourse.bass as bass
import concourse.tile as tile
from concourse import mybir

N_CORES = 8
N_FULL = 4194304
P = 128  # SBUF partitions

import os as _os

# Tunables (env-overridable for bench sweeps; defaults are the shipped config)
M = int(_os.environ.get("K_M", "512"))  # samples per partition per tile
BUFS = int(_os.environ.get("K_BUFS", "4"))

# Engine assignment for the multiply stages: "vector" or "gpsimd"
Z_ENGINE = _os.environ.get("K_Z_ENGINE", "vector")
W_ENGINE = _os.environ.get("K_W_ENGINE", "gpsimd")
OUT_ENGINE = _os.environ.get("K_OUT_ENGINE", "gpsimd")
# Number of out-stage components (0..5) computed on DVE instead of OUT_ENGINE,
# to balance the DVE and GpSimd pipeline stages. HW sweep: K=0 w/ wmul on
# DVE 137.1us, K=3 126.1us, K=2 144.3us (Pool saturates faster than the
# cost model predicts; K=3 is the hardware optimum).
OUT_SPLIT_K = int(_os.environ.get("K_OUT_SPLIT_K", "3"))
# Split the d/e input DMAs into two halves so compute starts earlier.
DMA_SPLIT = False
# Pairwise-add reductions instead of tensor_reduce (fewer DVE cycles).
PAIRWISE = False
# Input-DMA grouping: one d/e DMA covers DMA_GROUP compute sub-tiles
# (bigger transfers, fewer dispatches; compute still pipelines at m).
DMA_GROUP = int(_os.environ.get("K_DMA_GROUP", "1"))
IN_BUFS = int(_os.environ.get("K_IN_BUFS", "2"))  # bufs for the grouped input pools
# Ramp-up: split the first tile into RAMP sub-tiles of m/RAMP samples so the
# pipeline reaches steady state sooner (shorter first serial chain).
RAMP = int(_os.environ.get("K_RAMP", "2"))
# Register bias const tiles inside the TileContext (Tile-tracked deps)
# instead of pre-TC memsets + an extra all-engine barrier.
BIAS_IN_TC = False
# Engine that issues the out DMA ("sync" = SP ring, "scalar" = ACT ring).
OUT_DMA = _os.environ.get("K_OUT_DMA", "scalar")
# Use the single-op ~18-bit reciprocal instead of the 2-op ~22-bit one.
RECIP_FAST = bool(int(_os.environ.get("K_RECIP_FAST", "1")))
# Split the 6-way d reduction: Pool adds the two halves (3m elems), DVE
# reduces the remaining [m,3] — balances DVE (pipeline pole) against Pool.
POOL_REDUCE6 = bool(int(_os.environ.get("K_POOL_REDUCE6", "0")))
# Software-pipelined emission: engines run in program order, so emitting
# front(t) [reduce6, zmul, tanh, exp] then back(t-1) [reduce5, recip, wmul,
# outmul, store] keeps the DVE from stalling at reduce5(t) waiting for the
# ACT tanh/exp round-trip of the same tile.
SW_PIPE = bool(int(_os.environ.get("K_SW_PIPE", "0")))

# test.py can flip this to get profile/exec-time back
TRACE = False
LAST = {}

# Diagnostic modes for bench_loop decomposition (never used by kernel()):
# SKIP_COMPUTE: issue only the DMAs (out DMA sends stale e tiles).
# SKIP_DMA: issue only compute (tiles hold stale SBUF data).
SKIP_COMPUTE = bool(int(_os.environ.get("K_SKIP_COMPUTE", "0")))
SKIP_DMA = bool(int(_os.environ.get("K_SKIP_DMA", "0")))


def build_bass(W: float, bvals, S: int, m: int = M, bufs: int = BUFS, repeats: int = 1):
    """Build the single-core SPMD program: d[S,6], e[S,5] -> out[S,5].

    repeats>1 wraps the whole tile loop in a hardware For_i so bench_loop.py
    can measure steady-state device time via the wall-clock slope over R.
    """
    assert S % (P * m) == 0, (S, P, m)
    T = S // (P * m)
    f32 = mybir.dt.float32
    mult = mybir.AluOpType.mult
    add = mybir.AluOpType.add
    X = mybir.AxisListType.X
    ACT = mybir.ActivationFunctionType

    nc = bacc.Bacc("TRN2", debug=False, num_devices=N_CORES)

    # Register the bias values as const APs so activation(bias=<float>) works.
    for i, v in enumerate(dict.fromkeys(float(x) for x in bvals)):
        t_c = nc.alloc_sbuf_tensor(f"const-bias-{i}", [P, 1], f32)
        nc.gpsimd.memset(t_c.ap(), v)
        nc.const_aps.aps[(f32, v)] = t_c.ap()
    nc.all_engine_barrier()

    d_ap = nc.dram_tensor("d", [S, 6], f32, kind="ExternalInput").ap()
    e_ap = nc.dram_tensor("e", [S, 5], f32, kind="ExternalInput").ap()
    o_ap = nc.dram_tensor("out", [S, 5], f32, kind="ExternalOutput").ap()

    # [T, P, m*c] views; per partition the data is one contiguous DRAM run.
    d_v = d_ap.rearrange("(t p m) c -> t p (m c)", t=T, p=P, m=m)
    e_v = e_ap.rearrange("(t p m) c -> t p (m c)", t=T, p=P, m=m)
    o_v = o_ap.rearrange("(t p m) c -> t p (m c)", t=T, p=P, m=m)

    z_eng = {"vector": nc.vector, "gpsimd": nc.gpsimd}[Z_ENGINE]
    w_eng = {"vector": nc.vector, "gpsimd": nc.gpsimd}[W_ENGINE]
    out_eng = {"vector": nc.vector, "gpsimd": nc.gpsimd}[OUT_ENGINE]
    out_dma_eng = {"sync": nc.sync, "scalar": nc.scalar}[OUT_DMA]

    g = DMA_GROUP
    assert T % g == 0
    if g > 1:
        # grouped views: sample idx = ((tb*P + p)*g + sub)*m + j
        d_vg = d_ap.rearrange("(tb p n) c -> tb p (n c)", tb=T // g, p=P, n=m * g)
        e_vg = e_ap.rearrange("(tb p n) c -> tb p (n c)", tb=T // g, p=P, n=m * g)
        o_vg = o_ap.rearrange(
            "(tb p g m) c -> tb p g (m c)", tb=T // g, p=P, g=g, m=m
        )

    with tile.TileContext(nc) as tc, ExitStack() as ctx:
        if BIAS_IN_TC:
            cpool = ctx.enter_context(tc.tile_pool(name="cpool", bufs=1))
            for i, v in enumerate(dict.fromkeys(float(x) for x in bvals)):
                ct = cpool.tile([P, 1], f32, tag=f"bias{i}")
                nc.gpsimd.memset(ct[:], v)
                nc.const_aps.aps[(f32, v)] = ct[:]

        dpool = ctx.enter_context(
            tc.tile_pool(name="dpool", bufs=IN_BUFS if g > 1 else bufs)
        )
        epool = ctx.enter_context(
            tc.tile_pool(name="epool", bufs=IN_BUFS if g > 1 else bufs)
        )
        zpool = ctx.enter_context(tc.tile_pool(name="zpool", bufs=bufs))
        small = ctx.enter_context(tc.tile_pool(name="small", bufs=bufs))

        def emit_front(dt_, et, o_dst, mm):
            """Stage A of one sub-tile: d-reduce, z=s*e, tanh, exp."""
            ev = et.rearrange("p (m c) -> p m c", c=5)
            s_t = small.tile([P, mm], f32, tag="s")
            dv3 = dt_.rearrange("p (m c) -> p m c", c=6)
            if POOL_REDUCE6:
                h3 = small.tile([P, 3 * mm], f32, tag="h3")
                h3v = h3[:].rearrange("p (m c) -> p m c", c=3)
                nc.gpsimd.tensor_tensor(
                    out=h3v, in0=dv3[:, :, 0:3], in1=dv3[:, :, 3:6], op=add
                )
                nc.vector.tensor_reduce(out=s_t[:], in_=h3v, axis=X, op=add)
            else:
                nc.vector.tensor_reduce(out=s_t[:], in_=dv3, axis=X, op=add)

            z = zpool.tile([P, 5 * mm], f32, tag="z")
            zv = z[:].rearrange("p (m c) -> p m c", c=5)
            s_b = s_t[:].unsqueeze(-1).broadcast_to([P, mm, 5])
            z_eng.tensor_tensor(out=zv, in0=s_b, in1=ev, op=mult)

            for c in range(5):
                nc.scalar.activation(
                    out=zv[:, :, c],
                    in_=zv[:, :, c],
                    func=ACT.Tanh,
                    bias=float(bvals[c]),
                    scale=float(W),
                )
            nc.scalar.activation(out=z[:], in_=z[:], func=ACT.Exp)
            return (z, zv, et, ev, o_dst, mm)

        def emit_back(st):
            """Stage B: a-reduce, reciprocal, w=a*e, out=w*r, store."""
            z, zv, et, ev, o_dst, mm = st
            dnm = small.tile([P, mm], f32, tag="dnm")
            nc.vector.tensor_reduce(out=dnm[:], in_=zv, axis=X, op=add)
            r = small.tile([P, mm], f32, tag="r")
            if RECIP_FAST:
                # ~51 ULP (≈18 bits): orders of magnitude inside the 2e-2 gate.
                nc.vector.reciprocal_approx_fast(out=r[:], in_=dnm[:])
            else:
                scr = small.tile([P, mm], f32, tag="scr")
                nc.vector.reciprocal_approx_accurate(
                    out=r[:], in_=dnm[:], scratch=scr[:]
                )

            w_eng.tensor_tensor(out=et, in0=z[:], in1=et, op=mult)
            r_b = r[:].unsqueeze(-1).broadcast_to([P, mm, 5])
            k = OUT_SPLIT_K
            if k > 0:
                nc.vector.tensor_tensor(
                    out=zv[:, :, :k], in0=ev[:, :, :k], in1=r_b[:, :, :k], op=mult
                )
                out_eng.tensor_tensor(
                    out=zv[:, :, k:], in0=ev[:, :, k:], in1=r_b[:, :, k:], op=mult
                )
            else:
                out_eng.tensor_tensor(out=zv, in0=ev, in1=r_b, op=mult)
            if not SKIP_DMA:
                out_dma_eng.dma_start(out=o_dst, in_=z[:])

        pending = []  # front-emitted sub-tiles awaiting their back half

        def emit(dt_, et, o_dst, mm):
            """Compute + store one sub-tile of mm samples/partition."""
            if SKIP_COMPUTE:
                nc.sync.dma_start(out=o_dst, in_=et)
                return
            st = emit_front(dt_, et, o_dst, mm)
            pending.append(st)
            if not SW_PIPE or len(pending) > 1:
                emit_back(pending.pop(0))

        if SKIP_DMA:
            assert g == 1 and RAMP == 1
            for _ in range(bufs):
                dt0 = dpool.tile([P, 6 * m], f32, tag="dpool")
                nc.vector.memset(dt0[:], 0.0)
                et0 = epool.tile([P, 5 * m], f32, tag="epool")
                nc.vector.memset(et0[:], 0.0)

        rep_ctx = tc.For_i(0, repeats) if repeats > 1 else nullcontext()
        with rep_ctx:
          for t in range(T):
            bt, sub = divmod(t, g)
            if g > 1:
                if sub == 0:
                    dbig = dpool.tile([P, 6 * m * g], f32)
                    nc.sync.dma_start(out=dbig[:], in_=d_vg[bt])
                    ebig = epool.tile([P, 5 * m * g], f32)
                    nc.sync.dma_start(out=ebig[:], in_=e_vg[bt])
                emit(
                    dbig[:, sub * 6 * m : (sub + 1) * 6 * m],
                    ebig[:, sub * 5 * m : (sub + 1) * 5 * m],
                    o_vg[bt][:, sub, :],
                    m,
                )
            elif t == 0 and RAMP > 1:
                mr = m // RAMP
                for k in range(RAMP):
                    dk = dpool.tile([P, 6 * mr], f32, tag="dpool")
                    nc.sync.dma_start(
                        out=dk[:], in_=d_v[0][:, k * 6 * mr : (k + 1) * 6 * mr]
                    )
                    ek = epool.tile([P, 5 * mr], f32, tag="epool")
                    nc.sync.dma_start(
                        out=ek[:], in_=e_v[0][:, k * 5 * mr : (k + 1) * 5 * mr]
                    )
                    emit(
                        dk[:], ek[:], o_v[0][:, k * 5 * mr : (k + 1) * 5 * mr], mr
                    )
            else:
                dt_tile = dpool.tile([P, 6 * m], f32, tag="dpool")
                et_tile = epool.tile([P, 5 * m], f32, tag="epool")
                if not SKIP_DMA:
                    nc.sync.dma_start(out=dt_tile[:], in_=d_v[t])
                    nc.sync.dma_start(out=et_tile[:], in_=e_v[t])
                emit(dt_tile[:], et_tile[:], o_v[t], m)
          while pending:  # drain the software pipeline inside the iteration
              emit_back(pending.pop(0))

    # Legalize: split multi-wait instructions (HW allows 1 wait/inst).
    nc.compile()
    return nc


def kernel(d, e, W, b):
    from concourse.bass_utils import run_bass_kernel_spmd

    d = np.ascontiguousarray(d, dtype=np.float32)
    e = np.ascontiguousarray(e, dtype=np.float32)
    n = d.shape[0]
    assert n % N_CORES == 0
    s = n // N_CORES

    nc = build_bass(float(np.asarray(W).reshape(-1)[0]), np.asarray(b).tolist(), s)

    in_maps = [
        {"d": d[i * s : (i + 1) * s], "e": e[i * s : (i + 1) * s]}
        for i in range(N_CORES)
    ]
    res = run_bass_kernel_spmd(nc, in_maps, list(range(N_CORES)), trace=TRACE)
    LAST["results"] = res
    out = np.concatenate([res.results[i]["out"] for i in range(N_CORES)], axis=0)
    return out.astype(np.float32)

